# revision 1
# baseline (speedup 1.0000x reference)
"""Dynamic per-pixel depthwise 3x3 conv (DYDConv2d) on 8 Trainium2 cores.

Full-tensor contract:
    input : (8, 64, 128, 128) f32
    weight: (8, 64, 3, 3, 128, 128) f32   -- one 3x3 filter per (b, c, oh, ow)
    out   : (8, 64, 128, 128) f32
    out[b,c,oh,ow] = sum_{i,j} xpad[b,c,oh+i,ow+j] * weight[b,c,i,j,oh,ow]
    (stride 1, pad 1)

Sharding: data-parallel over batch B=8 -> one sample per NeuronCore.

fp16 end-to-end on device (harness tolerance is 2e-2; fp16 gives ~1e-3):
halves the dominant ~38 MB/core weight stream to ~19 MB AND doubles DVE
tensor_tensor throughput (2x_1P perf mode needs 16-bit dtype, unit stride,
4B-aligned operands).

Per-core layout: 128 SBUF partitions = (channel c) x (H-half hf), p=c*2+hf.
Each partition holds TWO 66x130 fp16 slabs of its half-image: slab0 is the
zero-padded slab, slab1 the same shifted left one column.  The 3x3 taps are
then free-dim views at EVEN element offsets (kw=0 -> slab0 col0, kw=1 ->
slab1 col0, kw=2 -> slab0 col2), so every DVE operand stays 4B-aligned and
the multiply runs in 2x mode.  Host casts inputs to fp16 and assembles the
slabs; device output is fp16, upcast on host.
"""

import numpy as np

import concourse.bacc as bacc
import concourse.mybir as mybir
from concourse.bass_utils import run_bass_kernel_spmd
from concourse.tile import TileContext

B, C, H, W = 8, 64, 128, 128
KH, KW = 3, 3
HALF = H // 2  # rows per half-image (one partition group)
SLAB_R, SLAB_C = HALF + 2, W + 2  # 66 x 130 padded slab per partition

RT = 16   # output rows per chunk (per half)
GRP = 3   # taps per weight-DMA group

_F16 = mybir.dt.float16

# Number of x slabs shipped per partition.  2 = extra column-shifted copy so
# every tap's column offset is even (4B-aligned -> DVE 2x mode on all 9
# multiplies).  1 = single slab; the three kw=1 taps read at odd offsets and
# drop to 1x, but the x DMA halves.
NSLABS = 2

# wrap-mode x buffer sizes: xw0 = 66 rows x 128 flat; xw1 = 2 lead zeros +
# per-row rotate-left-1 of xw0 (see make_xw)
XW_ROWS = HALF + 2
XW0_LEN = XW_ROWS * W
XW_TOT = 2 * XW0_LEN + 2

# tap kw -> (slab index, column offset)
_TAPCOL = {
    2: {0: (0, 0), 1: (1, 0), 2: (0, 2)},
    1: {0: (0, 0), 1: (0, 1), 2: (0, 2)},
}[NSLABS]


def _emit(nc, tc, xs, w, o, rep=1, rt=None, wgroup="tap", mode="full",
          xmode="slab2", sched="chain"):
    rt = RT if rt is None else rt
    if wgroup in ("all", "kh3"):
        wv = w  # already [128, 9, HALF, W] (host pre-arranged)
    else:
        wv = w.rearrange("c kh kw (hf r) ww -> c hf kh kw r ww", hf=2)
    ov = o.rearrange("c (hf r) ww -> (c hf) r ww", hf=2)

    with tc.tile_pool(name="work", bufs=2) as pool:
        for _r in range(rep):
            if xmode == "wrap":
                xbuf = pool.tile([128, XW_TOT], _F16, name="xbuf")
                nc.scalar.dma_start(out=xbuf[:], in_=xs[:])
            else:
                nsl = NSLABS if xmode == "slab2" else 1
                xbuf = pool.tile([128, nsl, SLAB_R, SLAB_C], _F16, name="xbuf")
                nc.scalar.dma_start(
                    out=xbuf[:].rearrange("p s r cc -> p (s r cc)"), in_=xs[:]
                )
            _emit_pass(nc, pool, xbuf, wv, ov, rt, wgroup, mode, xmode, sched)


def _emit_pass(nc, pool, xbuf, wv, ov, rt, wgroup, mode="full", xmode="slab2",
               sched="chain"):
    xc = None

    def xtap(i, j, r0):
        if xmode == "wrap":
            # flat 4B-aligned slices of xw0 / rotated xw1 (see make_xw)
            if j == 1:
                base = (r0 + i) * W            # xw0
            elif j == 0:
                base = XW0_LEN + (r0 + i) * W  # xw1, lead pad absorbs -2
            else:
                base = XW0_LEN + 2 + (r0 + i) * W
            return xbuf[:, base : base + rt * W]
        if xmode == "copy3":
            return xc[:, j, i : i + rt, :]
        s, col = _TAPCOL[j]
        return xbuf[:, s, r0 + i : r0 + i + rt, col : col + W]

    for k in range(HALF // rt):
        r0 = k * rt
        if mode != "dma" and sched != "tree":
            acc = pool.tile([128, rt, W], _F16, name="acc")
            tmp = pool.tile([128, rt, W], _F16, name="tmp", bufs=1)
            if xmode == "copy3":
                # three contiguous column-shifted x copies on the (idle) ACT
                # engine: makes every DVE multiply operand a flat unit-stride
                # 4B-aligned fp16 stream -> 2x perf mode
                xc = pool.tile([128, KW, rt + 2, W], _F16, name="xc")
                for kw in range(KW):
                    nc.scalar.copy(
                        out=xc[:, kw],
                        in_=xbuf[:, 0, r0 : r0 + rt + 2, kw : kw + W],
                    )
        first = True

        if wgroup == "all":
            wt = pool.tile([128, KH * KW, rt, W], _F16, name="wall")
            if mode != "compute":
                nc.sync.dma_start(out=wt[:], in_=wv[:, :, r0 : r0 + rt, :])
            else:
                # token write so the tile counts as written; DVE reads the
                # (mostly uninitialized) tile at full size
                nc.sync.dma_start(
                    out=wt[:, :, 0:1, :], in_=wv[:, :, r0 : r0 + 1, :]
                )
            taps = [divmod(t, KW) + (wt[:, t],) for t in range(KH * KW)]
        elif wgroup == "kh3":
            # three 3-tap DMAs per chunk from the partition-major weight
            taps = []
            for kh in range(KH):
                wt = pool.tile([128, KW, rt, W], _F16, name="wk3")
                if mode != "compute":
                    nc.sync.dma_start(
                        out=wt[:],
                        in_=wv[:, kh * KW : (kh + 1) * KW, r0 : r0 + rt, :],
                    )
                else:
                    nc.sync.dma_start(
                        out=wt[:, :, 0:1, :],
                        in_=wv[:, kh * KW : (kh + 1) * KW, r0 : r0 + 1, :],
                    )
                taps.extend((kh, kw, wt[:, kw]) for kw in range(KW))
        elif wgroup == "kh":
            # one 3-tap weight DMA per kh row: 3x bigger transfers
            taps = []
            for kh in range(KH):
                wt = pool.tile([128, KW, rt, W], _F16, name="wk")
                if mode != "compute":
                    nc.sync.dma_start(
                        out=wt[:], in_=wv[:, :, kh, :, r0 : r0 + rt, :]
                    )
                taps.extend((kh, kw, wt[:, kw]) for kw in range(KW))
        else:
            taps = []
            for t in range(KH * KW):
                i, j = divmod(t, KW)
                wt = pool.tile([128, rt, W], _F16, name=f"wg{t % GRP}")
                if mode != "compute":
                    nc.sync.dma_start(
                        out=wt[:], in_=wv[:, :, i, j, r0 : r0 + rt, :]
                    )
                taps.append((i, j, wt[:]))

        if mode == "dma":
            # no DVE work: out stream reads (written) xbuf instead of acc
            if xmode == "wrap":
                nc.scalar.dma_start(
                    out=ov[:, r0 : r0 + rt, :],
                    in_=xbuf[:, r0 * W : (r0 + rt) * W],
                )
            else:
                nc.scalar.dma_start(
                    out=ov[:, r0 : r0 + rt, :],
                    in_=xbuf[:, 0, r0 : r0 + rt, 1 : W + 1],
                )
            continue
        flat = xmode == "wrap"
        if sched == "tree":
            # 9 independent multiplies into separate product tiles, then a
            # pairwise in-place add tree: avoids the ~1us RAW-dependency
            # stall the DVE pays between adjacent producer->consumer ops
            prods = [
                pool.tile([128, rt * W], _F16, name=f"pr{t}",
                          bufs=2 if t == 0 else 1)
                for t in range(KH * KW)
            ]
            for t, (i, j, wtap) in enumerate(taps):
                wop = wtap.rearrange("p r w -> p (r w)") if flat else wtap
                nc.vector.tensor_tensor(
                    prods[t][:], xtap(i, j, r0), wop, mybir.AluOpType.mult
                )
            for dst, src_ in ((0, 1), (2, 3), (4, 5), (6, 7),
                              (0, 2), (4, 6), (0, 4), (0, 8)):
                nc.vector.tensor_tensor(
                    prods[dst][:], prods[dst][:], prods[src_][:],
                    mybir.AluOpType.add,
                )
            nc.scalar.dma_start(out=ov[:, r0 : r0 + rt, :], in_=prods[0][:])
            continue
        for i, j, wtap in taps:
            wop = wtap.rearrange("p r w -> p (r w)") if flat else wtap
            aop = acc[:].rearrange("p r w -> p (r w)") if flat else acc[:]
            top = tmp[:].rearrange("p r w -> p (r w)") if flat else tmp[:]
            if first:
                nc.vector.tensor_tensor(
                    aop, xtap(i, j, r0), wop, mybir.AluOpType.mult
                )
                first = False
            else:
                nc.vector.tensor_tensor(
                    top, xtap(i, j, r0), wop, mybir.AluOpType.mult
                )
                nc.vector.tensor_tensor(aop, aop, top, mybir.AluOpType.add)
        nc.scalar.dma_start(out=ov[:, r0 : r0 + rt, :], in_=acc[:])


# adopted default configuration (fastest benched variant)
DEFAULTS = dict(rt=16, wgroup="all", xmode="wrap")


def make_inputs(x_one, w_one):
    """Per-sample host input map for the adopted default configuration."""
    if DEFAULTS["xmode"] == "wrap":
        return {"xs": make_xw(x_one), "w": make_w_wrap(w_one)}
    if DEFAULTS["wgroup"] in ("all", "kh3"):
        return {"xs": make_slab(x_one), "w": make_w_all(w_one)}
    return {"xs": make_slab(x_one), "w": make_w(w_one)}


def build_program(rep=1, rt=None, wgroup=None, mode="full", xmode=None,
                  sched=None, **_ignored):
    rt = DEFAULTS["rt"] if rt is None else rt
    wgroup = DEFAULTS["wgroup"] if wgroup is None else wgroup
    xmode = DEFAULTS["xmode"] if xmode is None else xmode
    sched = DEFAULTS.get("sched", "chain") if sched is None else sched
    nc = bacc.Bacc(
        "TRN2",
        target_bir_lowering=False,
        debug=False,
        enable_asserts=False,
        num_devices=8,
    )
    if xmode == "wrap":
        xs = nc.dram_tensor("xs", [128, XW_TOT], _F16, kind="ExternalInput").ap()
    else:
        nsl = NSLABS if xmode == "slab2" else 1
        xs = nc.dram_tensor(
            "xs", [128, nsl * SLAB_R * SLAB_C], _F16, kind="ExternalInput"
        ).ap()
    if wgroup in ("all", "kh3"):
        w = nc.dram_tensor(
            "w", [128, KH * KW, HALF, W], _F16, kind="ExternalInput"
        ).ap()
    else:
        w = nc.dram_tensor("w", [C, KH, KW, H, W], _F16, kind="ExternalInput").ap()
    o = nc.dram_tensor("o", [C, H, W], _F16, kind="ExternalOutput").ap()
    with TileContext(nc) as tc:
        _emit(nc, tc, xs, w, o, rep=rep, rt=rt, wgroup=wgroup, mode=mode,
              xmode=xmode, sched=sched)
    nc.compile()
    return nc


def make_slab(x_one, nslabs=None):
    """Host-side slab(s) for one sample: [64,128,128] f32 -> [128, nslabs*66*130] fp16.

    Partition p = c*2 + hf holds rows hf*64-1 .. hf*64+64 of channel c
    (zero-padded at the image border) in a 66x130 col-padded layout;
    slab s=0 unshifted, s=1 shifted left one column (for 4B-aligned kw=1
    taps).
    """
    nslabs = NSLABS if nslabs is None else nslabs
    slab = np.zeros((C, 2, nslabs, SLAB_R, SLAB_C), dtype=np.float16)  # (c, hf, s, r, col)
    x16 = x_one.astype(np.float16)
    # half 0: slab rows 1..65 <- x rows 0..64 (row 0 stays zero: top pad)
    slab[:, 0, 0, 1 : HALF + 2, 1 : W + 1] = x16[:, 0 : HALF + 1, :]
    # half 1: slab rows 0..64 <- x rows 63..127 (row 65 stays zero: bottom pad)
    slab[:, 1, 0, 0 : HALF + 1, 1 : W + 1] = x16[:, HALF - 1 : H, :]
    if nslabs == 2:
        slab[:, :, 1, :, 0 : SLAB_C - 1] = slab[:, :, 0, :, 1:SLAB_C]
    return slab.reshape(128, nslabs * SLAB_R * SLAB_C)


def make_w(w_one):
    """Host-side fp16 cast of one sample's weights: [64,3,3,128,128]."""
    return np.ascontiguousarray(w_one.astype(np.float16))


def make_xw(x_one):
    """Wrap-mode x layout: [128, 2*66*128 + 2] fp16 per sample.

    Per partition p = c*2 + hf:
      xw0 = rows -1..64 of the half-image (border rows zero), 128 wide, flat;
      xw1 = 2 zero lead elems, then per-row rotate-left-by-1 of xw0.
    All 9 taps become flat 4B-aligned slices; the wrap artifacts hit only
    weight entries zeroed by make_w_wrap (whose true contribution is x-pad*w
    = 0), so the result is exact.
    """
    x16 = x_one.astype(np.float16)
    xw0 = np.zeros((C, 2, XW_ROWS, W), dtype=np.float16)
    xw0[:, 0, 1:] = x16[:, 0 : HALF + 1, :]      # half 0: rows -1..64
    xw0[:, 1, : XW_ROWS - 1] = x16[:, HALF - 1 : H, :]  # half 1: rows 63..128
    xw0f = xw0.reshape(C, 2, XW0_LEN)
    xw1f = np.zeros_like(xw0f)                   # flat shift-left-by-1
    xw1f[:, :, : XW0_LEN - 1] = xw0f[:, :, 1:]
    out = np.zeros((C, 2, XW_TOT), dtype=np.float16)
    out[:, :, :XW0_LEN] = xw0f
    # lead element 1 is read by tap (kh=0, kw=0) at (oh=0, ow=1): it must be
    # xw0[0, 0] (the halo row's first elem; zero only for the top half)
    out[:, :, XW0_LEN + 1] = xw0f[:, :, 0]
    out[:, :, XW0_LEN + 2 :] = xw1f
    return out.reshape(128, XW_TOT)


def make_w_wrap(w_one):
    """make_w_all layout with the two wrap-artifact weight columns zeroed:
    w[c, kh, 0, :, 0] and w[c, kh, 2, :, 127] multiply x-padding (true
    contribution 0), and in the wrap layout they would touch garbage."""
    w16 = w_one.astype(np.float16).copy()
    w16[:, :, 0, :, 0] = 0
    w16[:, :, 2, :, 127] = 0
    w6 = w16.reshape(C, KH, KW, 2, HALF, W)
    arr = w6.transpose(0, 3, 1, 2, 4, 5)
    return np.ascontiguousarray(arr.reshape(128, KH * KW, HALF, W))


def make_w_all(w_one):
    """Partition-major weight layout for wgroup='all': [128, 9, HALF, W].

    Partition p = c*2 + hf holds its half-image's 9 tap maps contiguously,
    so one chunk's weights load in a single 3-dim-clean DMA.
    """
    w6 = w_one.astype(np.float16).reshape(C, KH, KW, 2, HALF, W)
    arr = w6.transpose(0, 3, 1, 2, 4, 5)  # (c, hf, kh, kw, HALF, W)
    return np.ascontiguousarray(arr.reshape(128, KH * KW, HALF, W))


_CACHE = {}


def _spot_check(out, input, weight, n=16):
    """Max rel err of `out` vs host reference on n random output rows.

    Cheap (n*9 row FMAs on host) guard against rare transient device
    faults; fp16 end-to-end lands ~2e-4 here, garbage lands ~1.
    """
    rng = np.random.default_rng(0)
    xpad = np.pad(input, ((0, 0), (0, 0), (1, 1), (1, 1)))
    worst = 0.0
    for b, c, r in zip(
        rng.integers(0, B, n), rng.integers(0, C, n), rng.integers(0, H, n)
    ):
        exp = np.zeros(W, np.float32)
        for i in range(KH):
            for j in range(KW):
                exp += xpad[b, c, r + i, j : j + W] * weight[b, c, i, j, r, :]
        scale = max(float(np.abs(exp).max()), 1.0)
        worst = max(worst, float(np.abs(out[b, c, r] - exp).max()) / scale)
    return worst


def kernel(input, weight, _trace=False):
    input = np.asarray(input, dtype=np.float32)
    weight = np.asarray(weight, dtype=np.float32)
    assert input.shape == (B, C, H, W), input.shape
    assert weight.shape == (B, C, KH, KW, H, W), weight.shape

    if "nc" not in _CACHE:
        _CACHE["nc"] = build_program()
    nc = _CACHE["nc"]

    in_maps = [make_inputs(input[b], weight[b]) for b in range(B)]
    for attempt in range(3):
        res = run_bass_kernel_spmd(
            nc, in_maps, core_ids=list(range(B)), trace=_trace
        )
        _CACHE["last_result"] = res
        out = np.stack([res.results[b]["o"] for b in range(B)], axis=0)
        out = out.astype(np.float32)
        if _spot_check(out, input, weight) < 8e-3:
            break
    return out



# revision 10
# speedup vs baseline: 1.0829x; 1.0829x over previous
"""Dynamic per-pixel depthwise 3x3 conv (DYDConv2d) on 8 Trainium2 cores.

Full-tensor contract:
    input : (8, 64, 128, 128) f32
    weight: (8, 64, 3, 3, 128, 128) f32   -- one 3x3 filter per (b, c, oh, ow)
    out   : (8, 64, 128, 128) f32
    out[b,c,oh,ow] = sum_{i,j} xpad[b,c,oh+i,ow+j] * weight[b,c,i,j,oh,ow]
    (stride 1, pad 1)

Sharding: data-parallel over batch B=8 -> one sample per NeuronCore.

fp16 end-to-end on device (harness tolerance 2e-2; fp16 gives ~1e-3).

Per-core layout: 128 SBUF partitions = (channel c) x (H-half hf), p=c*2+hf.
Each partition holds one 66x130 zero-padded fp16 slab of its half-image;
the 9 taps are free-dim views of the slab (row offset i, col offset j).

The 17 elementwise plane-combines per chunk (9 mults + 8 adds) are split
across engines so none is the sole bottleneck:
  - DVE   : most taps (tensor_tensor at 2x perf mode)
  - GPSIMD: `gps_taps` trailing taps (own mult/add chain into tile g)
  - merge : acc += g, either on DVE or as an SBUF->SBUF CCE-accumulate DMA
Optionally the trailing `fp8_taps` weight planes ship as fp8(e4m3) in HBM
(halving their DMA bytes) and are upcast fp8->fp16 by the otherwise-idle
ACT engine before use.
"""

import numpy as np

import concourse.bacc as bacc
import concourse.mybir as mybir
from concourse.bass_utils import run_bass_kernel_spmd
from concourse.tile import TileContext

B, C, H, W = 8, 64, 128, 128
KH, KW = 3, 3
HALF = H // 2                      # rows per half-image (one partition group)
SLAB_R, SLAB_C = HALF + 2, W + 2   # 66 x 130 padded slab per partition

_F16 = mybir.dt.float16
_F8 = mybir.dt.float8e4

# tap t -> (row offset i, col offset j)
_TAPS = [(t // KW, t % KW) for t in range(KH * KW)]

# adopted default configuration
DEFAULTS = dict(rt=16, gps_taps=2, merge="dve", fp8_taps=0)
# trailing knobs adopted from the TimelineSim sweep
BEST = dict(out_q="sync", gps_pattern=(2, 3, 2, 3))


def _emit(nc, tc, xs, w, w8, o, rep=1, rt=16, gps_taps=2, merge="dve",
          fp8_taps=0, mode="full", wt_bufs=2, xbuf_bufs=2, acc_bufs=2,
          g_bufs=2, out_q="scalar", gps_pattern=None, dma_rows=None,
          out_rows=None):
    """Emit `rep` full passes.

    rt       : compute-chunk rows; dma_rows/out_rows: weight-load / output-
               store granularities (multiples of rt; default rt).
    Tap split: taps [0, 9-n_gps) on DVE, trailing n_gps taps on GPSIMD
    (per-compute-chunk override via gps_pattern). The trailing fp8_taps
    taps load as fp8 and are upcast by ACT.
    """
    dma_rows = rt if dma_rows is None else dma_rows
    out_rows = rt if out_rows is None else out_rows
    n_f16 = KH * KW - fp8_taps
    ov = o.rearrange("c (hf r) ww -> (c hf) r ww", hf=2)
    out_eng = {"scalar": nc.scalar, "sync": nc.sync}[out_q]
    assert dma_rows % rt == 0 and out_rows % rt == 0
    assert HALF % dma_rows == 0 and HALF % out_rows == 0

    with tc.tile_pool(name="work", bufs=2) as pool:
        for _r in range(rep):
            xbuf = pool.tile([128, SLAB_R, SLAB_C], _F16, name="xbuf",
                             bufs=xbuf_bufs)
            nc.scalar.dma_start(
                out=xbuf[:].rearrange("p r cc -> p (r cc)"), in_=xs[:]
            )
            wt = acc = w8t = None
            for k in range(HALF // rt):
                r0 = k * rt
                n_gps = (gps_pattern[k % len(gps_pattern)]
                         if gps_pattern else gps_taps)
                n_dve = KH * KW - n_gps
                # ---- weight loads (dma_rows granularity) --------------
                if r0 % dma_rows == 0:
                    d0 = r0
                    wt = pool.tile([128, KH * KW, dma_rows, W], _F16,
                                   name="wt", bufs=wt_bufs)
                    if mode != "compute":
                        if n_f16 > 0:
                            nc.sync.dma_start(
                                out=wt[:, :n_f16],
                                in_=w[:, :n_f16, d0 : d0 + dma_rows, :],
                            )
                        if fp8_taps > 0:
                            w8t = pool.tile(
                                [128, fp8_taps, dma_rows, W], _F8, name="w8t"
                            )
                            nc.sync.dma_start(
                                out=w8t[:],
                                in_=w8[:, :, d0 : d0 + dma_rows, :],
                            )
                    else:
                        nc.sync.dma_start(
                            out=wt[:, :, 0:1, :], in_=w[:, :, d0 : d0 + 1, :]
                        )
                s0 = r0 % dma_rows  # row offset within wt

                if mode != "compute" and fp8_taps > 0:
                    # ACT upcast fp8 -> fp16 (one op per tap plane per chunk)
                    for u in range(fp8_taps):
                        nc.scalar.copy(
                            out=wt[:, n_f16 + u, s0 : s0 + rt],
                            in_=w8t[:, u, s0 : s0 + rt],
                        )

                def xtap(t):
                    i, j = _TAPS[t]
                    return xbuf[:, r0 + i : r0 + i + rt, j : j + W]

                def wtap(t):
                    return wt[:, t, s0 : s0 + rt]

                if mode == "dma":
                    if (r0 + rt) % out_rows == 0:
                        o0 = r0 + rt - out_rows
                        out_eng.dma_start(
                            out=ov[:, o0 : o0 + out_rows, :],
                            in_=xbuf[:, o0 : o0 + out_rows, 1 : W + 1],
                        )
                    continue

                # ---- output accumulator (out_rows granularity) --------
                if r0 % out_rows == 0:
                    acc = pool.tile([128, out_rows, W], _F16, name="acc",
                                    bufs=acc_bufs)
                a0 = r0 % out_rows
                av = acc[:, a0 : a0 + rt]

                # ---- DVE chain: taps [0, n_dve) -----------------------
                tmp = pool.tile([128, rt, W], _F16, name="tmp", bufs=1)
                nc.vector.tensor_tensor(
                    av, xtap(0), wtap(0), mybir.AluOpType.mult
                )
                for t in range(1, n_dve):
                    nc.vector.tensor_tensor(
                        tmp[:], xtap(t), wtap(t), mybir.AluOpType.mult
                    )
                    nc.vector.tensor_tensor(
                        av, av, tmp[:], mybir.AluOpType.add
                    )

                # ---- GPSIMD chain: taps [n_dve, 9) --------------------
                if n_gps > 0:
                    g = pool.tile([128, rt, W], _F16, name="g", bufs=g_bufs)
                    gt = pool.tile([128, rt, W], _F16, name="gt", bufs=1)
                    nc.gpsimd.tensor_tensor(
                        g[:], xtap(n_dve), wtap(n_dve), mybir.AluOpType.mult
                    )
                    for t in range(n_dve + 1, KH * KW):
                        nc.gpsimd.tensor_tensor(
                            gt[:], xtap(t), wtap(t), mybir.AluOpType.mult
                        )
                        nc.gpsimd.tensor_tensor(
                            g[:], g[:], gt[:], mybir.AluOpType.add
                        )
                    if merge == "cce":
                        # SBUF->SBUF accumulate DMA: acc += g on the DMA
                        # engines' CCE adders (SWDGE path required).
                        nc.gpsimd.dma_start(
                            out=av, in_=g[:],
                            accum_op=mybir.AluOpType.add,
                        )
                    else:
                        nc.vector.tensor_tensor(
                            av, av, g[:], mybir.AluOpType.add
                        )

                if (r0 + rt) % out_rows == 0:
                    o0 = r0 + rt - out_rows
                    out_eng.dma_start(
                        out=ov[:, o0 : o0 + out_rows, :], in_=acc[:]
                    )


def build_program(rep=1, rt=None, gps_taps=None, merge=None, fp8_taps=None,
                  mode="full", wt_bufs=2, xbuf_bufs=2, acc_bufs=2, g_bufs=2,
                  out_q=None, gps_pattern=None, dma_rows=None,
                  out_rows=None, **_ignored):
    out_q = BEST["out_q"] if out_q is None else out_q
    gps_pattern = (BEST["gps_pattern"] if gps_pattern is None
                   else (gps_pattern or None))
    rt = DEFAULTS["rt"] if rt is None else rt
    gps_taps = DEFAULTS["gps_taps"] if gps_taps is None else gps_taps
    merge = DEFAULTS["merge"] if merge is None else merge
    fp8_taps = DEFAULTS["fp8_taps"] if fp8_taps is None else fp8_taps
    nc = bacc.Bacc(
        "TRN2",
        target_bir_lowering=False,
        debug=False,
        enable_asserts=False,
        num_devices=8,
    )
    xs = nc.dram_tensor(
        "xs", [128, SLAB_R * SLAB_C], _F16, kind="ExternalInput"
    ).ap()
    n_f16 = KH * KW - fp8_taps
    w = nc.dram_tensor(
        "w", [128, max(n_f16, 1), HALF, W], _F16, kind="ExternalInput"
    ).ap()
    if fp8_taps > 0:
        w8 = nc.dram_tensor(
            "w8", [128, fp8_taps, HALF, W], _F8, kind="ExternalInput"
        ).ap()
    else:
        w8 = None
    o = nc.dram_tensor("o", [C, H, W], _F16, kind="ExternalOutput").ap()
    with TileContext(nc) as tc:
        _emit(nc, tc, xs, w, w8, o, rep=rep, rt=rt, gps_taps=gps_taps,
              merge=merge, fp8_taps=fp8_taps, mode=mode, wt_bufs=wt_bufs,
              xbuf_bufs=xbuf_bufs, acc_bufs=acc_bufs, g_bufs=g_bufs,
              out_q=out_q, gps_pattern=gps_pattern, dma_rows=dma_rows,
              out_rows=out_rows)
    nc.compile()
    return nc


def make_slab(x_one):
    """Host-side slab for one sample: [64,128,128] f32 -> [128, 66*130] fp16.

    Partition p = c*2 + hf holds rows hf*64-1 .. hf*64+64 of channel c
    (zero-padded at the image border) in a 66x130 col-padded layout.
    """
    slab = np.zeros((C, 2, SLAB_R, SLAB_C), dtype=np.float16)
    x16 = x_one.astype(np.float16)
    # half 0: slab rows 1..65 <- x rows 0..64 (row 0 stays zero: top pad)
    slab[:, 0, 1 : HALF + 2, 1 : W + 1] = x16[:, 0 : HALF + 1, :]
    # half 1: slab rows 0..64 <- x rows 63..127 (row 65 stays zero: bottom pad)
    slab[:, 1, 0 : HALF + 1, 1 : W + 1] = x16[:, HALF - 1 : H, :]
    return slab.reshape(128, SLAB_R * SLAB_C)


def _w_perm(w_one):
    """[64,3,3,128,128] f32 -> [128, 9, HALF, W] f32 partition-major."""
    w6 = w_one.reshape(C, KH, KW, 2, HALF, W)
    arr = w6.transpose(0, 3, 1, 2, 4, 5)  # (c, hf, kh, kw, HALF, W)
    return arr.reshape(128, KH * KW, HALF, W)


def make_inputs(x_one, w_one, fp8_taps=None):
    fp8_taps = DEFAULTS["fp8_taps"] if fp8_taps is None else fp8_taps
    n_f16 = KH * KW - fp8_taps
    wp = _w_perm(w_one)
    out = {"xs": make_slab(x_one)}
    out["w"] = np.ascontiguousarray(wp[:, : max(n_f16, 1)].astype(np.float16))
    if fp8_taps > 0:
        from ml_dtypes import float8_e4m3

        out["w8"] = np.ascontiguousarray(
            wp[:, n_f16:].astype(float8_e4m3)
        )
    return out


_CACHE = {}


def _spot_check(out, input, weight, n=16):
    """Max rel err of `out` vs host reference on n random output rows."""
    rng = np.random.default_rng(0)
    xpad = np.pad(input, ((0, 0), (0, 0), (1, 1), (1, 1)))
    worst = 0.0
    for b, c, r in zip(
        rng.integers(0, B, n), rng.integers(0, C, n), rng.integers(0, H, n)
    ):
        exp = np.zeros(W, np.float32)
        for i in range(KH):
            for j in range(KW):
                exp += xpad[b, c, r + i, j : j + W] * weight[b, c, i, j, r, :]
        scale = max(float(np.abs(exp).max()), 1.0)
        worst = max(worst, float(np.abs(out[b, c, r] - exp).max()) / scale)
    return worst


def kernel(input, weight, _trace=False):
    input = np.asarray(input, dtype=np.float32)
    weight = np.asarray(weight, dtype=np.float32)
    assert input.shape == (B, C, H, W), input.shape
    assert weight.shape == (B, C, KH, KW, H, W), weight.shape

    if "nc" not in _CACHE:
        _CACHE["nc"] = build_program()
    nc = _CACHE["nc"]

    in_maps = [make_inputs(input[b], weight[b]) for b in range(B)]
    for attempt in range(3):
        res = run_bass_kernel_spmd(
            nc, in_maps, core_ids=list(range(B)), trace=_trace
        )
        _CACHE["last_result"] = res
        out = np.stack([res.results[b]["o"] for b in range(B)], axis=0)
        out = out.astype(np.float32)
        if _spot_check(out, input, weight) < 8e-3:
            break
    return out


# revision 22
# speedup vs baseline: 1.1847x; 1.0940x over previous
"""Dynamic per-pixel depthwise 3x3 conv (DYDConv2d) on 8 Trainium2 cores.

Full-tensor contract:
    input : (8, 64, 128, 128) f32
    weight: (8, 64, 3, 3, 128, 128) f32   -- one 3x3 filter per (b, c, oh, ow)
    out   : (8, 64, 128, 128) f32
    out[b,c,oh,ow] = sum_{i,j} xpad[b,c,oh+i,ow+j] * weight[b,c,i,j,oh,ow]
    (stride 1, pad 1)

Sharding: data-parallel over batch B=8 -> one sample per NeuronCore.

fp16 end-to-end on device (harness tolerance 2e-2; fp16 gives ~1e-3).

Per-core layout: 128 SBUF partitions = (channel c) x (H-half hf), p=c*2+hf.
Each partition holds one 66x130 zero-padded fp16 slab of its half-image;
the 9 taps are free-dim views of the slab (row offset i, col offset j).

Per 16-row chunk the DVE runs 9 independent tensor_tensor multiplies into
separate product tiles followed by a pairwise add tree ("tree" schedule) --
measured ~25% faster on silicon than the mult/add chain (no back-to-back
RAW dependencies, so the DVE pipeline stays full).  Weight chunks stream
on the sync (SP) HWDGE queue double-buffered; x loads once per pass and
outputs store on the same sync queue.

A/B-benched alternatives kept as knobs but OFF by default (all measured
slower on hardware): GPSIMD tap offload (SBUF-port contention with the
DVE), SBUF->SBUF CCE-accumulate merges (loads the DMA engines), fp8
weight planes (fails the 2e-2 tolerance: single large products dominate),
ACT-side shifted-slab alignment copies (no measurable effect).
"""

import numpy as np

import concourse.bacc as bacc
import concourse.mybir as mybir
from concourse.bass_utils import run_bass_kernel_spmd
from concourse.tile import TileContext

B, C, H, W = 8, 64, 128, 128
KH, KW = 3, 3
HALF = H // 2                      # rows per half-image (one partition group)
SLAB_R, SLAB_C = HALF + 2, W + 2   # 66 x 130 padded slab per partition

_F16 = mybir.dt.float16
_F8 = mybir.dt.float8e4

# tap t -> (row offset i, col offset j)
_TAPS = [(t // KW, t % KW) for t in range(KH * KW)]

# adopted default configuration (fastest measured on hardware)
DEFAULTS = dict(rt=16, gps_taps=0, merge="dve", fp8_taps=0)
BEST = dict(out_q="sync", gps_pattern=(0,), sched="tree")


def _emit(nc, tc, xs, w, w8, o, rep=1, rt=16, gps_taps=2, merge="dve",
          fp8_taps=0, mode="full", wt_bufs=2, xbuf_bufs=2, acc_bufs=2,
          g_bufs=2, out_q="scalar", gps_pattern=None, dma_rows=None,
          out_rows=None, xshift=False, tail_eng="gps", sched="chain",
          w_chunk_major=False, pr_bufs=1):
    """Emit `rep` full passes.

    rt       : compute-chunk rows; dma_rows/out_rows: weight-load / output-
               store granularities (multiples of rt; default rt).
    Tap split: taps [0, 9-n_gps) on DVE, trailing n_gps taps on GPSIMD
    (per-compute-chunk override via gps_pattern). The trailing fp8_taps
    taps load as fp8 and are upcast by ACT.
    """
    dma_rows = rt if dma_rows is None else dma_rows
    out_rows = rt if out_rows is None else out_rows
    n_f16 = KH * KW - fp8_taps
    ov = o.rearrange("c (hf r) ww -> (c hf) r ww", hf=2)
    out_eng = {"scalar": nc.scalar, "sync": nc.sync}[out_q]
    assert dma_rows % rt == 0 and out_rows % rt == 0
    assert HALF % dma_rows == 0 and HALF % out_rows == 0

    with tc.tile_pool(name="work", bufs=2) as pool:
        for _r in range(rep):
            xbuf = pool.tile([128, SLAB_R, SLAB_C], _F16, name="xbuf",
                             bufs=xbuf_bufs)
            nc.scalar.dma_start(
                out=xbuf[:].rearrange("p r cc -> p (r cc)"), in_=xs[:]
            )
            if xshift:
                # ACT-side column-shifted slab so the kw=1 taps read at
                # 4B-aligned offsets (keeps DVE 2x perf mode on silicon).
                xb2 = pool.tile([128, SLAB_R, SLAB_C], _F16, name="xb2",
                                bufs=xbuf_bufs)
                nc.scalar.copy(
                    out=xb2[:, :, 0 : SLAB_C - 2], in_=xbuf[:, :, 1 : SLAB_C - 1]
                )
            wt = acc = w8t = None
            for k in range(HALF // rt):
                r0 = k * rt
                n_gps = (gps_pattern[k % len(gps_pattern)]
                         if gps_pattern else gps_taps)
                n_dve = KH * KW - n_gps
                # ---- weight loads (dma_rows granularity) --------------
                if r0 % dma_rows == 0:
                    d0 = r0
                    wt = pool.tile([128, KH * KW, dma_rows, W], _F16,
                                   name="wt", bufs=wt_bufs)
                    if mode != "compute":
                        if n_f16 > 0:
                            if w_chunk_major:
                                assert dma_rows == rt
                                nc.sync.dma_start(
                                    out=wt[:, :n_f16],
                                    in_=w[:, k, :n_f16],
                                )
                            else:
                                nc.sync.dma_start(
                                    out=wt[:, :n_f16],
                                    in_=w[:, :n_f16, d0 : d0 + dma_rows, :],
                                )
                        if fp8_taps > 0:
                            w8t = pool.tile(
                                [128, fp8_taps, dma_rows, W], _F8, name="w8t"
                            )
                            nc.sync.dma_start(
                                out=w8t[:],
                                in_=w8[:, :, d0 : d0 + dma_rows, :],
                            )
                    else:
                        assert not w_chunk_major
                        nc.sync.dma_start(
                            out=wt[:, :, 0:1, :], in_=w[:, :, d0 : d0 + 1, :]
                        )
                s0 = r0 % dma_rows  # row offset within wt

                if mode != "compute" and fp8_taps > 0:
                    # ACT upcast fp8 -> fp16 (one op per tap plane per chunk)
                    for u in range(fp8_taps):
                        nc.scalar.copy(
                            out=wt[:, n_f16 + u, s0 : s0 + rt],
                            in_=w8t[:, u, s0 : s0 + rt],
                        )

                def xtap(t):
                    i, j = _TAPS[t]
                    if xshift and j == 1:
                        return xb2[:, r0 + i : r0 + i + rt, 0:W]
                    return xbuf[:, r0 + i : r0 + i + rt, j : j + W]

                def wtap(t):
                    return wt[:, t, s0 : s0 + rt]

                if mode == "dma":
                    if (r0 + rt) % out_rows == 0:
                        o0 = r0 + rt - out_rows
                        out_eng.dma_start(
                            out=ov[:, o0 : o0 + out_rows, :],
                            in_=xbuf[:, o0 : o0 + out_rows, 1 : W + 1],
                        )
                    continue

                # ---- output accumulator (out_rows granularity) --------
                if r0 % out_rows == 0:
                    acc = pool.tile([128, out_rows, W], _F16, name="acc",
                                    bufs=acc_bufs)
                a0 = r0 % out_rows
                av = acc[:, a0 : a0 + rt]

                # ---- DVE chain: taps [0, n_dve) -----------------------
                if sched == "tree":
                    # independent product tiles + pairwise add tree
                    prods = [
                        pool.tile([128, rt, W], _F16, name=f"pr{t}",
                                  bufs=pr_bufs)
                        for t in range(n_dve)
                    ]
                    for t in range(n_dve):
                        nc.vector.tensor_tensor(
                            prods[t][:], xtap(t), wtap(t), mybir.AluOpType.mult
                        )
                    live = list(range(n_dve))
                    while len(live) > 2:
                        nxt = []
                        for i in range(0, len(live) - 1, 2):
                            a, b = live[i], live[i + 1]
                            nc.vector.tensor_tensor(
                                prods[a][:], prods[a][:], prods[b][:],
                                mybir.AluOpType.add,
                            )
                            nxt.append(a)
                        if len(live) % 2:
                            nxt.append(live[-1])
                        live = nxt
                    if len(live) == 2:
                        nc.vector.tensor_tensor(
                            av, prods[live[0]][:], prods[live[1]][:],
                            mybir.AluOpType.add,
                        )
                    else:
                        nc.vector.tensor_tensor(
                            av, prods[live[0]][:], prods[live[0]][:],
                            mybir.AluOpType.bypass,
                        )
                else:
                    tmp = pool.tile([128, rt, W], _F16, name="tmp", bufs=1)
                    nc.vector.tensor_tensor(
                        av, xtap(0), wtap(0), mybir.AluOpType.mult
                    )
                    for t in range(1, n_dve):
                        nc.vector.tensor_tensor(
                            tmp[:], xtap(t), wtap(t), mybir.AluOpType.mult
                        )
                        nc.vector.tensor_tensor(
                            av, av, tmp[:], mybir.AluOpType.add
                        )

                # ---- tail chain: taps [n_dve, 9) ----------------------
                if n_gps > 0:
                    teng = nc.gpsimd if tail_eng == "gps" else nc.vector
                    g = pool.tile([128, rt, W], _F16, name="g", bufs=g_bufs)
                    gt = pool.tile([128, rt, W], _F16, name="gt", bufs=1)
                    teng.tensor_tensor(
                        g[:], xtap(n_dve), wtap(n_dve), mybir.AluOpType.mult
                    )
                    for t in range(n_dve + 1, KH * KW):
                        teng.tensor_tensor(
                            gt[:], xtap(t), wtap(t), mybir.AluOpType.mult
                        )
                        teng.tensor_tensor(
                            g[:], g[:], gt[:], mybir.AluOpType.add
                        )
                    if merge == "cce":
                        # SBUF->SBUF accumulate DMA: acc += g on the DMA
                        # engines' CCE adders (SWDGE path required).
                        nc.gpsimd.dma_start(
                            out=av, in_=g[:],
                            accum_op=mybir.AluOpType.add,
                        )
                    else:
                        nc.vector.tensor_tensor(
                            av, av, g[:], mybir.AluOpType.add
                        )

                if (r0 + rt) % out_rows == 0:
                    o0 = r0 + rt - out_rows
                    out_eng.dma_start(
                        out=ov[:, o0 : o0 + out_rows, :], in_=acc[:]
                    )


def build_program(rep=1, rt=None, gps_taps=None, merge=None, fp8_taps=None,
                  mode="full", wt_bufs=2, xbuf_bufs=2, acc_bufs=2, g_bufs=2,
                  out_q=None, gps_pattern=None, dma_rows=None,
                  out_rows=None, xshift=None, tail_eng=None, sched=None,
                  w_chunk_major=None, pr_bufs=1, **_ignored):
    w_chunk_major = (BEST.get("w_chunk_major", False)
                     if w_chunk_major is None else w_chunk_major)
    xshift = BEST.get("xshift", False) if xshift is None else xshift
    tail_eng = BEST.get("tail_eng", "gps") if tail_eng is None else tail_eng
    sched = BEST.get("sched", "chain") if sched is None else sched
    out_q = BEST["out_q"] if out_q is None else out_q
    gps_pattern = (BEST["gps_pattern"] if gps_pattern is None
                   else (gps_pattern or None))
    rt = DEFAULTS["rt"] if rt is None else rt
    gps_taps = DEFAULTS["gps_taps"] if gps_taps is None else gps_taps
    merge = DEFAULTS["merge"] if merge is None else merge
    fp8_taps = DEFAULTS["fp8_taps"] if fp8_taps is None else fp8_taps
    nc = bacc.Bacc(
        "TRN2",
        target_bir_lowering=False,
        debug=False,
        enable_asserts=False,
        num_devices=8,
    )
    xs = nc.dram_tensor(
        "xs", [128, SLAB_R * SLAB_C], _F16, kind="ExternalInput"
    ).ap()
    n_f16 = KH * KW - fp8_taps
    if w_chunk_major:
        wr = DEFAULTS["rt"] if rt is None else rt
        w = nc.dram_tensor(
            "w", [128, HALF // wr, max(n_f16, 1), wr, W], _F16,
            kind="ExternalInput"
        ).ap()
    else:
        w = nc.dram_tensor(
            "w", [128, max(n_f16, 1), HALF, W], _F16, kind="ExternalInput"
        ).ap()
    if fp8_taps > 0:
        w8 = nc.dram_tensor(
            "w8", [128, fp8_taps, HALF, W], _F8, kind="ExternalInput"
        ).ap()
    else:
        w8 = None
    o = nc.dram_tensor("o", [C, H, W], _F16, kind="ExternalOutput").ap()
    with TileContext(nc) as tc:
        _emit(nc, tc, xs, w, w8, o, rep=rep, rt=rt, gps_taps=gps_taps,
              merge=merge, fp8_taps=fp8_taps, mode=mode, wt_bufs=wt_bufs,
              xbuf_bufs=xbuf_bufs, acc_bufs=acc_bufs, g_bufs=g_bufs,
              out_q=out_q, gps_pattern=gps_pattern, dma_rows=dma_rows,
              out_rows=out_rows, xshift=xshift, tail_eng=tail_eng,
              sched=sched, w_chunk_major=w_chunk_major, pr_bufs=pr_bufs)
    nc.compile()
    return nc


def make_slab(x_one):
    """Host-side slab for one sample: [64,128,128] f32 -> [128, 66*130] fp16.

    Partition p = c*2 + hf holds rows hf*64-1 .. hf*64+64 of channel c
    (zero-padded at the image border) in a 66x130 col-padded layout.
    """
    slab = np.zeros((C, 2, SLAB_R, SLAB_C), dtype=np.float16)
    x16 = x_one.astype(np.float16)
    # half 0: slab rows 1..65 <- x rows 0..64 (row 0 stays zero: top pad)
    slab[:, 0, 1 : HALF + 2, 1 : W + 1] = x16[:, 0 : HALF + 1, :]
    # half 1: slab rows 0..64 <- x rows 63..127 (row 65 stays zero: bottom pad)
    slab[:, 1, 0 : HALF + 1, 1 : W + 1] = x16[:, HALF - 1 : H, :]
    return slab.reshape(128, SLAB_R * SLAB_C)


def _w_perm(w_one):
    """[64,3,3,128,128] f32 -> [128, 9, HALF, W] f32 partition-major."""
    w6 = w_one.reshape(C, KH, KW, 2, HALF, W)
    arr = w6.transpose(0, 3, 1, 2, 4, 5)  # (c, hf, kh, kw, HALF, W)
    return arr.reshape(128, KH * KW, HALF, W)


def make_inputs(x_one, w_one, fp8_taps=None, w_chunk_major=None, rt=None):
    fp8_taps = DEFAULTS["fp8_taps"] if fp8_taps is None else fp8_taps
    w_chunk_major = (BEST.get("w_chunk_major", False)
                     if w_chunk_major is None else w_chunk_major)
    rt = DEFAULTS["rt"] if rt is None else rt
    n_f16 = KH * KW - fp8_taps
    wp = _w_perm(w_one)
    out = {"xs": make_slab(x_one)}
    wf = wp[:, : max(n_f16, 1)].astype(np.float16)
    if w_chunk_major:
        # [128, T, HALF, W] -> [128, HALF//rt, T, rt, W]
        T = wf.shape[1]
        wf = wf.reshape(128, T, HALF // rt, rt, W).transpose(0, 2, 1, 3, 4)
    out["w"] = np.ascontiguousarray(wf)
    if fp8_taps > 0:
        from ml_dtypes import float8_e4m3

        out["w8"] = np.ascontiguousarray(
            wp[:, n_f16:].astype(float8_e4m3)
        )
    return out


_CACHE = {}


def _spot_check(out, input, weight, n=16):
    """Max rel err of `out` vs host reference on n random output rows."""
    rng = np.random.default_rng(0)
    xpad = np.pad(input, ((0, 0), (0, 0), (1, 1), (1, 1)))
    worst = 0.0
    for b, c, r in zip(
        rng.integers(0, B, n), rng.integers(0, C, n), rng.integers(0, H, n)
    ):
        exp = np.zeros(W, np.float32)
        for i in range(KH):
            for j in range(KW):
                exp += xpad[b, c, r + i, j : j + W] * weight[b, c, i, j, r, :]
        scale = max(float(np.abs(exp).max()), 1.0)
        worst = max(worst, float(np.abs(out[b, c, r] - exp).max()) / scale)
    return worst


def kernel(input, weight, _trace=False):
    input = np.asarray(input, dtype=np.float32)
    weight = np.asarray(weight, dtype=np.float32)
    assert input.shape == (B, C, H, W), input.shape
    assert weight.shape == (B, C, KH, KW, H, W), weight.shape

    if "nc" not in _CACHE:
        _CACHE["nc"] = build_program()
    nc = _CACHE["nc"]

    in_maps = [make_inputs(input[b], weight[b]) for b in range(B)]
    for attempt in range(3):
        res = run_bass_kernel_spmd(
            nc, in_maps, core_ids=list(range(B)), trace=_trace
        )
        _CACHE["last_result"] = res
        out = np.stack([res.results[b]["o"] for b in range(B)], axis=0)
        out = out.astype(np.float32)
        if _spot_check(out, input, weight) < 8e-3:
            break
    return out
